# revision 1
# baseline (speedup 1.0000x reference)
"""GroupWiseLinear Trainium2 kernel.

out[b, c] = dot(W[0, c, :], x[b, group_of[c], :]) + bias[0, c], then a final
class-permutation gather, for two independent branches (co / cl).

Sharding: 8 cores = 2 branches x 4 class-quarters (1024 classes each, all 64
batches per core).  The ragged group segments of each core's class range are
split/padded on host into uniform 64-column "slots" so that every core runs
the SAME instruction stream (SPMD) on different data:

  - xt:  [128, S*4*64]  per-slot x^T (H-major), replicated per slot
  - wt:  [128, 4*S*64]  W^T (H-major), zero-padded to slot layout
  - bz:  [1, S*64]      bias, zero-padded to slot layout
  - out: [64, S*64]     padded per-core output (batch-major)

Device work per slot: 4 K-chunk matmuls (x stationary [128,64], W moving) that
accumulate into PSUM, plus a rank-1 ones-matmul adding the bias.  Host
"unshard" places each core's real columns into the final permuted output.
"""

import ml_dtypes
import numpy as np

import concourse.bacc as bacc
import concourse.tile as tile
from concourse import mybir
from concourse.bass_utils import run_bass_kernel_spmd

B = 64          # batch
H = 512         # hidden
NC_CLS = 4096   # classes per branch
NQ = 4          # class-quarters per branch
QCLS = NC_CLS // NQ
KC = H // 128   # contraction chunks

_cache = {}


def _build_shards(co_group_of, cl_group_of):
    """Per (branch, quarter): list of slots (group, cls_start, width<=64)."""
    shards = []
    for go in (co_group_of, cl_group_of):
        go = np.asarray(go).astype(np.int64)
        for q in range(NQ):
            c0, c1 = q * QCLS, (q + 1) * QCLS
            slots = []
            i = c0
            while i < c1:
                g = go[i]
                j = i
                while j < c1 and go[j] == g:
                    j += 1
                for s in range(i, j, 64):
                    slots.append((int(g), s, min(64, j - s)))
                i = j
            shards.append(slots)
    return shards


def _program(S, dt=mybir.dt.bfloat16):
    """Build the uniform SPMD Bass program for S slots per core."""
    nc = bacc.Bacc("TRN2", target_bir_lowering=False, debug=False, num_devices=8)
    xt_d = nc.dram_tensor("xt", [128, S * KC * 64], dt, kind="ExternalInput")
    wt_d = nc.dram_tensor("wt", [128, KC, S * 64], dt, kind="ExternalInput")
    bz_d = nc.dram_tensor("bz", [1, S * 64], dt, kind="ExternalInput")
    nhalf = ((S * 64 + 511) // 512 + 1) // 2
    o_d = nc.dram_tensor("o", [128, 512 * nhalf], mybir.dt.float32, kind="ExternalOutput")

    ntiles = (S * 64 + 511) // 512

    with tile.TileContext(nc) as tc:
        with (
            tc.tile_pool(name="xp", bufs=4 * ntiles) as xp,
            tc.tile_pool(name="wp", bufs=ntiles * KC) as wp,
            tc.tile_pool(name="cp", bufs=1) as cp,
            tc.tile_pool(name="op", bufs=ntiles) as op,
            tc.tile_pool(name="ps", bufs=min(ntiles, 8), space="PSUM") as ps,
        ):
            ones = cp.tile([1, 64], dt)
            nc.gpsimd.memset(ones[:], 1.0)
            bz = cp.tile([1, S * 64], dt)
            nc.scalar.dma_start(bz[:], bz_d[:])

            ohs = []
            for t in range(ntiles):
                s_lo = t * 8
                s_hi = min(S, s_lo + 8)
                nsl = s_hi - s_lo
                tw = nsl * 64

                xt = xp.tile([128, nsl * KC * 64], dt)
                nc.sync.dma_start(xt[:], xt_d[:, s_lo * KC * 64 : s_hi * KC * 64])
                wt = wp.tile([128, KC, tw], dt)
                nc.scalar.dma_start(wt[:], wt_d[:, :, s_lo * 64 : s_hi * 64])

                acc = ps.tile([64, 512], mybir.dt.float32)
                for sl in range(nsl):
                    for k in range(KC):
                        nc.tensor.matmul(
                            acc[0:64, sl * 64 : (sl + 1) * 64],
                            xt[:, (sl * KC + k) * 64 : (sl * KC + k + 1) * 64],
                            wt[:, k, sl * 64 : (sl + 1) * 64],
                            start=(k == 0),
                            stop=False,
                        )
                    nc.tensor.matmul(
                        acc[0:64, sl * 64 : (sl + 1) * 64],
                        ones[0:1, 0:64],
                        bz[0:1, (s_lo + sl) * 64 : (s_lo + sl + 1) * 64],
                        start=False,
                        stop=True,
                    )

                if t % 2 == 0:
                    oh = op.tile([128, 512], mybir.dt.float32)
                    ohs.append(oh)
                oh = ohs[t // 2]
                r0 = 64 * (t % 2)
                nc.vector.tensor_copy(oh[r0 : r0 + 64, 0:tw], acc[0:64, 0:tw])
                if t % 2 == 1 or t == ntiles - 1:
                    h = t // 2
                    eng = nc.sync if h % 2 == 0 else nc.scalar
                    eng.dma_start(o_d[:, h * 512 : (h + 1) * 512], oh[:])

    nc.compile()
    return nc


def _host_prep(x, W, bias, slots, S, goff):
    """Build xt/wt/bz arrays for one core."""
    nsl = len(slots)
    groups = np.array([g for g, _, _ in slots], np.int64)
    # xt: [128, S*KC*64]; col = s*(KC*64) + k*64 + b
    xg = x[:, goff + groups, :]                      # [B, nsl, H]
    xt = np.zeros((128, S * KC * 64), ml_dtypes.bfloat16)
    xt[:, : nsl * KC * 64] = (
        xg.reshape(B, nsl, KC, 128).transpose(3, 1, 2, 0).reshape(128, nsl * KC * 64)
    )
    # wt: [128, KC*S*64]; col = k*(S*64) + s*64 + j
    Wp = np.zeros((S * 64, H), ml_dtypes.bfloat16)
    bz = np.zeros((1, S * 64), ml_dtypes.bfloat16)
    for s, (g, cst, wdt) in enumerate(slots):
        Wp[s * 64 : s * 64 + wdt] = W[cst : cst + wdt]
        bz[0, s * 64 : s * 64 + wdt] = bias[cst : cst + wdt]
    wt = Wp.reshape(S * 64, KC, 128).transpose(2, 1, 0).reshape(128, KC * S * 64)
    return {"xt": xt, "wt": np.ascontiguousarray(wt).reshape(128, KC, S * 64), "bz": bz}


def kernel(x, co_W, cl_W, co_b, cl_b, co_group_of, cl_group_of, co_index,
           cl_index, group_len, _iters=1, _return_raw=False):
    x = np.asarray(x, np.float32)
    G = int(group_len)
    shards = _build_shards(co_group_of, cl_group_of)
    S = max(len(s) for s in shards)

    key = ("v5bf16", S)
    if key not in _cache:
        _cache[key] = _program(S)
    nc = _cache[key]

    Ws = (np.asarray(co_W, np.float32)[0], np.asarray(cl_W, np.float32)[0])
    bs = (np.asarray(co_b, np.float32)[0], np.asarray(cl_b, np.float32)[0])
    in_maps = []
    for k in range(8):
        bi, q = divmod(k, NQ)
        in_maps.append(_host_prep(x, Ws[bi], bs[bi], shards[k], S, bi * G))

    res = run_bass_kernel_spmd(nc, in_maps, list(range(8)))

    outs = []
    for bi, index in ((0, co_index), (1, cl_index)):
        full = np.empty((B, NC_CLS), np.float32)
        for q in range(NQ):
            slots = shards[bi * NQ + q]
            src = np.empty(QCLS, np.int64)
            for s, (g, cst, wdt) in enumerate(slots):
                src[cst - q * QCLS : cst - q * QCLS + wdt] = np.arange(
                    s * 64, s * 64 + wdt
                )
            oarr = res.results[bi * NQ + q]["o"]
            ntiles = (S * 64 + 511) // 512
            flat = np.empty((B, S * 64), np.float32)
            for t in range(ntiles):
                s_lo, s_hi = t * 8, min(S, t * 8 + 8)
                tw = (s_hi - s_lo) * 64
                r0 = 64 * (t % 2)
                flat[:, s_lo * 64 : s_lo * 64 + tw] = oarr[
                    r0 : r0 + 64, (t // 2) * 512 : (t // 2) * 512 + tw
                ]
            full[:, q * QCLS : (q + 1) * QCLS] = flat[:, src]
        outs.append(full[:, np.asarray(index).astype(np.int64)])
    return outs[0], outs[1]



# revision 5
# speedup vs baseline: 1.1772x; 1.1772x over previous
"""GroupWiseLinear Trainium2 kernel (v1: paired-stationary, unpadded W).

out[b, c] = dot(W[0, c, :], x[b, group_of[c], :]) + bias[0, c], then a final
class-permutation gather, for two independent branches (co / cl).

Sharding: 8 cores = 2 branches x 4 class-quarters (1024 classes each, all 64
batches per core).

Device scheme per core (one uniform SPMD program):
  - The core's ragged group segments are split into items (<=256 classes) and
    paired big+small.  Each pair becomes a "bin": the two groups' x^T chunks
    sit side by side as a [128, 128] stationary operand (batches 0-63 in PE
    columns 0-63 / PSUM partitions 0-63 for the first group, 64-127 for the
    second).  The bin's W columns stream unpadded through one matmul per
    128-deep K chunk (4 total), accumulating [128, N_bin] in PSUM.
  - Every output column is valid in exactly one half (top if its class
    belongs to the pair's first group, bottom otherwise); the host picks the
    right half, adds bias, and applies the final class permutation.
  - Cross-core uniformity: bins are sorted by width per core and the per-
    position width profile (max over cores) is baked into the program; cores
    zero-pad W (and x) up to the profile, so all 8 cores run the same
    instruction stream on different data.

HBM traffic per core ~2.4 MB (vs ~4.5 MB for the slot-padded baseline):
W is loaded exactly once, x once per group occurrence, output in bf16.
"""

import ml_dtypes
import numpy as np

import concourse.bacc as bacc
import concourse.tile as tile
from concourse import mybir
from concourse.bass_utils import run_bass_kernel_spmd

B = 64          # batch
H = 512         # hidden
NC_CLS = 4096   # classes per branch
NQ = 4          # class-quarters per branch
QCLS = NC_CLS // NQ
KC = H // 128   # contraction chunks
MAX_ITEM = 256  # max classes per item so a pair fits a 512-col PSUM bank
PSUM_W = 512    # PSUM bank width in fp32 columns
N_SLICES = 4    # pipelined load slices

_cache = {}


def _segments(go, c0, c1):
    """Contiguous (group, start, width) runs of go[c0:c1]."""
    segs = []
    i = c0
    while i < c1:
        g = int(go[i])
        j = i
        while j < c1 and go[j] == g:
            j += 1
        segs.append((g, i, j - i))
        i = j
    return segs


def _make_items(segs):
    """Split segments into items of width <= MAX_ITEM."""
    items = []
    for g, s, w in segs:
        for o in range(0, w, MAX_ITEM):
            items.append((g, s + o, min(MAX_ITEM, w - o)))
    return items


def _pair_items(items):
    """Pair big+small so pair widths are near-uniform. Returns list of
    (itemA, itemB-or-None)."""
    order = sorted(items, key=lambda t: -t[2])
    pairs = []
    lo, hi = 0, len(order) - 1
    while lo <= hi:
        if lo == hi:
            pairs.append((order[lo], None))
        else:
            pairs.append((order[lo], order[hi]))
        lo += 1
        hi -= 1
    pairs.sort(key=lambda p: -(p[0][2] + (p[1][2] if p[1] else 0)))
    return pairs


def _plan(co_group_of, cl_group_of):
    """Host-side plan: per-core bins + shared width profile + PSUM packing."""
    gos = (np.asarray(co_group_of).astype(np.int64),
           np.asarray(cl_group_of).astype(np.int64))
    core_pairs = []
    for k in range(8):
        bi, q = divmod(k, NQ)
        segs = _segments(gos[bi], q * QCLS, (q + 1) * QCLS)
        core_pairs.append(_pair_items(_make_items(segs)))

    nbins = max(len(p) for p in core_pairs)
    profile = []
    for j in range(nbins):
        w = 0
        for p in core_pairs:
            if j < len(p):
                a, b = p[j]
                w = max(w, a[2] + (b[2] if b else 0))
        profile.append(min(PSUM_W, (w + 7) // 8 * 8))

    # Pack bins (descending widths) into PSUM banks of PSUM_W fp32 columns.
    ptile, poff = [], []
    t, off = 0, 0
    for w in profile:
        if off + w > PSUM_W:
            t += 1
            off = 0
        ptile.append(t)
        poff.append(off)
        off += w
    ntiles = t + 1

    # Slice boundaries over bins, balanced by (xt + wt) bytes per bin.
    weights = [4 * (128 + w) for w in profile]
    tot = sum(weights)
    bounds = [0]
    acc = 0
    for j, w in enumerate(weights):
        acc += w
        if acc >= tot * len(bounds) / N_SLICES and len(bounds) < N_SLICES:
            bounds.append(j + 1)
    while len(bounds) < N_SLICES:
        bounds.append(nbins)
    bounds.append(nbins)

    return core_pairs, profile, ptile, poff, ntiles, bounds


def _program(profile, ptile, poff, ntiles, bounds, dt=mybir.dt.bfloat16):
    nbins = len(profile)
    prefn = np.concatenate([[0], np.cumsum(profile)]).astype(int)
    totn = int(prefn[-1])

    nc = bacc.Bacc("TRN2", target_bir_lowering=False, debug=False, num_devices=8)
    xt_d = nc.dram_tensor("xt", [128, nbins * KC * 128], dt, kind="ExternalInput")
    wt_d = nc.dram_tensor("wt", [128, KC * totn], dt, kind="ExternalInput")
    o_d = nc.dram_tensor("o", [128, ntiles * PSUM_W], dt, kind="ExternalOutput")

    with tile.TileContext(nc) as tc:
        with (
            tc.tile_pool(name="xp", bufs=N_SLICES) as xp,
            tc.tile_pool(name="wp", bufs=N_SLICES) as wp,
            tc.tile_pool(name="op", bufs=1) as op,
            tc.tile_pool(name="ps", bufs=1, space="PSUM") as ps,
        ):
            pst = [
                ps.tile([128, PSUM_W], mybir.dt.float32, name=f"ps{t}")
                for t in range(ntiles)
            ]
            done = 0  # psum tiles copied out so far
            for s in range(N_SLICES):
                b0, b1 = bounds[s], bounds[s + 1]
                if b0 == b1:
                    continue
                xts = xp.tile([128, (b1 - b0) * KC * 128], dt)
                nc.sync.dma_start(xts[:], xt_d[:, b0 * KC * 128 : b1 * KC * 128])
                wts = wp.tile([128, KC * (prefn[b1] - prefn[b0])], dt)
                nc.scalar.dma_start(
                    wts[:], wt_d[:, KC * prefn[b0] : KC * prefn[b1]]
                )
                for j in range(b0, b1):
                    n = profile[j]
                    xoff = (j - b0) * KC * 128
                    woff = KC * (prefn[j] - prefn[b0])
                    acc = pst[ptile[j]][:, poff[j] : poff[j] + n]
                    for k in range(KC):
                        nc.tensor.matmul(
                            acc,
                            xts[:, xoff + k * 128 : xoff + (k + 1) * 128],
                            wts[:, woff + k * n : woff + (k + 1) * n],
                            start=(k == 0),
                            stop=(k == KC - 1),
                        )
                    # flush any psum tile whose bins are all emitted
                    while done < ntiles and (
                        j == nbins - 1 or ptile[j + 1] > done
                    ):
                        ot = op.tile([128, PSUM_W], dt, name=f"ot{done}")
                        nc.vector.tensor_copy(ot[:], pst[done][:])
                        eng = nc.sync if done % 2 == 0 else nc.scalar
                        eng.dma_start(
                            o_d[:, done * PSUM_W : (done + 1) * PSUM_W], ot[:]
                        )
                        done += 1

    nc.compile()
    return nc


def _host_prep(x, W, pairs, profile, goff, dt=ml_dtypes.bfloat16):
    """Build xt/wt for one core plus the (class -> (col, half)) map."""
    nbins = len(profile)
    prefn = np.concatenate([[0], np.cumsum(profile)]).astype(int)
    totn = int(prefn[-1])

    xt = np.zeros((128, nbins * KC * 128), dt)
    wt = np.zeros((128, KC * totn), dt)
    cols = np.empty(QCLS, np.int64)   # per local class: column in o
    half = np.empty(QCLS, np.int64)   # 0 = rows 0-63, 1 = rows 64-127
    cbase = None

    # x[:, g, :] -> [128, KC, 64] h-major chunks, computed lazily per group
    xg_cache = {}

    def xgT(g):
        if g not in xg_cache:
            xg_cache[g] = np.ascontiguousarray(
                x[:, goff + g, :].astype(np.float32)
                .reshape(B, KC, 128).transpose(2, 1, 0)
            ).astype(dt)  # [128, KC, 64]
        return xg_cache[g]

    for j, (ia, ib) in enumerate(pairs):
        n = profile[j]
        # stationary halves
        for hidx, it in ((0, ia), (1, ib)):
            if it is None:
                continue
            g, _, _ = it
            xs = xgT(g)  # [128, KC, 64]
            for k in range(KC):
                c0 = j * KC * 128 + k * 128 + hidx * 64
                xt[:, c0 : c0 + 64] = xs[:, k, :]
        # streamed W columns: itemA's classes then itemB's
        coff = 0
        for hidx, it in ((0, ia), (1, ib)):
            if it is None:
                continue
            g, s, w = it
            if cbase is None:
                cbase = s - (s % QCLS)
            wTk = W[s : s + w].astype(np.float32).reshape(w, KC, 128)
            for k in range(KC):
                wcol = KC * prefn[j] + k * n + coff
                wt[:, wcol : wcol + w] = wTk[:, k, :].T
            lc = np.arange(s, s + w) % QCLS
            cols[lc] = prefn[j] + coff + np.arange(w)
            half[lc] = hidx
            coff += w
    return {"xt": xt, "wt": wt}, cols, half


def kernel(x, co_W, cl_W, co_b, cl_b, co_group_of, cl_group_of, co_index,
           cl_index, group_len, _return_raw=False):
    x = np.asarray(x, np.float32)
    G = int(group_len)
    core_pairs, profile, ptile, poff, ntiles, bounds = _plan(
        co_group_of, cl_group_of
    )

    key = ("v1", tuple(profile), tuple(bounds), ntiles)
    if key not in _cache:
        _cache.clear()
        _cache[key] = _program(profile, ptile, poff, ntiles, bounds)
    nc = _cache[key]

    prefn = np.concatenate([[0], np.cumsum(profile)]).astype(int)
    Ws = (np.asarray(co_W, np.float32)[0], np.asarray(cl_W, np.float32)[0])
    bs = (np.asarray(co_b, np.float32)[0], np.asarray(cl_b, np.float32)[0])
    in_maps, colmaps = [], []
    for k in range(8):
        bi, q = divmod(k, NQ)
        im, cols, half = _host_prep(x, Ws[bi], core_pairs[k], profile, bi * G)
        in_maps.append(im)
        colmaps.append((cols, half))

    res = run_bass_kernel_spmd(nc, in_maps, list(range(8)))

    # o layout: [128, ntiles*PSUM_W]; bin j occupies columns
    # ptile[j]*PSUM_W + poff[j] ... + profile[j]
    bin_col0 = np.array([ptile[j] * PSUM_W + poff[j] for j in range(len(profile))])
    outs = []
    for bi, index in ((0, co_index), (1, cl_index)):
        full = np.empty((B, NC_CLS), np.float32)
        for q in range(NQ):
            core = bi * NQ + q
            cols, half = colmaps[core]
            # translate prefn-space columns to o-space
            j_of = np.searchsorted(prefn, cols, side="right") - 1
            ocol = bin_col0[j_of] + (cols - prefn[j_of])
            oarr = np.asarray(res.results[core]["o"], np.float32)
            o3 = oarr.reshape(2, B, ntiles * PSUM_W)
            vals = o3[half, :, ocol]           # [QCLS, B]
            full[:, q * QCLS : (q + 1) * QCLS] = vals.T
        full += bs[bi][None, :]
        outs.append(full[:, np.asarray(index).astype(np.int64)])
    return outs[0], outs[1]


# revision 6
# speedup vs baseline: 1.2363x; 1.0502x over previous
"""GroupWiseLinear Trainium2 kernel (v2: exact-fill paired bins).

out[b, c] = dot(W[0, c, :], x[b, group_of[c], :]) + bias[0, c], then a final
class-permutation gather, for two independent branches (co / cl).

Sharding: 8 cores = 2 branches x 4 class-quarters (1024 classes each, all 64
batches per core).  One uniform SPMD program runs on all cores.

Device scheme per core:
  - The quarter's ragged group segments are packed into B bins of exactly
    T=128 streamed W columns (last bins may be short).  A bin holds columns
    from at most TWO groups (possibly the same group twice): the two groups'
    x^T chunks sit side by side as a [128, 128] matmul stationary (batches
    0-63 -> PSUM partitions 0-63 for half A, 64-127 for half B).  The bin's
    W columns stream unpadded through one matmul per 128-deep K chunk (4),
    accumulating [128, T] in PSUM.
  - Every output column is valid in exactly one half; the host picks the
    half, adds bias, and applies the final class permutation.
  - W is loaded exactly once (zero padding only in short bins); x once per
    bin-half occurrence (~= once per group).  All tensors bf16 on the wire;
    fp32 accumulation in PSUM.
  - Pipeline: bins are grouped into slices; each slice has its own PSUM
    bank, xt/wt load DMAs (SP/Act engines), one PSUM->SBUF bf16 copy (DVE)
    and one output DMA.  Slice sizes taper (3,3,2,1 bins) so the tail after
    the last load is short.

HBM traffic per core ~2.5 MB vs ~4.5 MB for the slot-padded baseline.
"""

import ml_dtypes
import numpy as np

import concourse.bacc as bacc
import concourse.tile as tile
from concourse import mybir
from concourse.bass_utils import run_bass_kernel_spmd

B = 64          # batch
H = 512         # hidden
NC_CLS = 4096   # classes per branch
NQ = 4          # class-quarters per branch
QCLS = NC_CLS // NQ
KC = H // 128   # contraction chunks
T = 128         # streamed W columns per bin
PSUM_W = 512    # PSUM bank width in fp32 columns

_cache = {}


def _segments(go, c0, c1):
    segs = []
    i = c0
    while i < c1:
        g = int(go[i])
        j = i
        while j < c1 and go[j] == g:
            j += 1
        segs.append((g, i, j - i))
        i = j
    return segs


def _fill_bins(segs):
    """Pack segments into bins of exactly T columns from <=2 groups each.

    Returns list of bins; each bin is a list of (group, cls_start, width)
    pieces (1 or 2), total width <= T.  Splitting a segment is free for W
    and costs only a duplicated x slot."""
    pool = sorted([list(s) for s in segs], key=lambda s: s[2])  # asc width
    bins = []
    while pool:
        a = pool.pop()  # largest
        if a[2] >= T:
            piece = (a[0], a[1], T)
            rest = a[2] - T
            if rest:
                ins = [a[0], a[1] + T, rest]
                lo = 0
                while lo < len(pool) and pool[lo][2] < rest:
                    lo += 1
                pool.insert(lo, ins)
            bins.append([piece])
            continue
        need = T - a[2]
        # smallest item with width >= need gives an exact fill
        idx = None
        for i, it in enumerate(pool):
            if it[2] >= need:
                idx = i
                break
        if idx is not None:
            b = pool.pop(idx)
            piece = (b[0], b[1], need)
            rest = b[2] - need
            if rest:
                ins = [b[0], b[1] + need, rest]
                lo = 0
                while lo < len(pool) and pool[lo][2] < rest:
                    lo += 1
                pool.insert(lo, ins)
            bins.append([tuple(a), piece])
        elif pool:
            b = pool.pop()  # largest remaining, short bin
            bins.append([tuple(a), tuple(b)])
        else:
            bins.append([tuple(a)])
    bins.sort(key=lambda bn: -sum(p[2] for p in bn))
    return bins


def _slice_sizes(nbins):
    """Tapered slice sizes, e.g. 9 -> [3, 3, 2, 1]; each slice <= 4 bins so
    one PSUM bank covers it."""
    if nbins <= 2:
        return [nbins]
    sizes = [1, 2]
    rem = nbins - 3
    while rem > 0:
        sizes.append(min(3, rem))
        rem -= 3
    return sizes[::-1]


def _plan(co_group_of, cl_group_of):
    gos = (np.asarray(co_group_of).astype(np.int64),
           np.asarray(cl_group_of).astype(np.int64))
    core_bins = []
    for k in range(8):
        bi, q = divmod(k, NQ)
        segs = _segments(gos[bi], q * QCLS, (q + 1) * QCLS)
        core_bins.append(_fill_bins(segs))
    nbins = max(len(cb) for cb in core_bins)
    profile = []
    for j in range(nbins):
        w = max(
            (sum(p[2] for p in cb[j]) if j < len(cb) else 0)
            for cb in core_bins
        )
        profile.append((max(w, 8) + 7) // 8 * 8)
    sizes = _slice_sizes(nbins)
    bounds = [0]
    for s in sizes:
        bounds.append(bounds[-1] + s)
    return core_bins, profile, bounds


def _program(profile, bounds, dt=mybir.dt.bfloat16):
    nbins = len(profile)
    nsl = len(bounds) - 1
    prefn = np.concatenate([[0], np.cumsum(profile)]).astype(int)
    totn = int(prefn[-1])

    nc = bacc.Bacc("TRN2", target_bir_lowering=False, debug=False, num_devices=8)
    xt_d = nc.dram_tensor("xt", [128, nbins * KC * 128], dt, kind="ExternalInput")
    wt_d = nc.dram_tensor("wt", [128, KC * totn], dt, kind="ExternalInput")
    o_d = nc.dram_tensor("o", [128, totn], dt, kind="ExternalOutput")

    with tile.TileContext(nc) as tc:
        with (
            tc.tile_pool(name="xp", bufs=nsl) as xp,
            tc.tile_pool(name="wp", bufs=nsl) as wp,
            tc.tile_pool(name="op", bufs=1) as op,
            tc.tile_pool(name="ps", bufs=1, space="PSUM") as ps,
        ):
            for s in range(nsl):
                b0, b1 = bounds[s], bounds[s + 1]
                sw = int(prefn[b1] - prefn[b0])  # slice width in columns
                xts = xp.tile([128, (b1 - b0) * KC * 128], dt)
                nc.sync.dma_start(xts[:], xt_d[:, b0 * KC * 128 : b1 * KC * 128])
                wts = wp.tile([128, KC * sw], dt)
                nc.scalar.dma_start(
                    wts[:], wt_d[:, KC * prefn[b0] : KC * prefn[b1]]
                )
                acc = ps.tile([128, sw], mybir.dt.float32, name=f"ps{s}")
                for j in range(b0, b1):
                    n = profile[j]
                    xoff = (j - b0) * KC * 128
                    woff = KC * (prefn[j] - prefn[b0])
                    aoff = int(prefn[j] - prefn[b0])
                    for k in range(KC):
                        nc.tensor.matmul(
                            acc[:, aoff : aoff + n],
                            xts[:, xoff + k * 128 : xoff + (k + 1) * 128],
                            wts[:, woff + k * n : woff + (k + 1) * n],
                            start=(k == 0),
                            stop=(k == KC - 1),
                        )
                ot = op.tile([128, sw], dt, name=f"ot{s}")
                nc.vector.tensor_copy(ot[:], acc[:])
                eng = nc.sync if s % 2 == 0 else nc.scalar
                eng.dma_start(o_d[:, prefn[b0] : prefn[b1]], ot[:])

    nc.compile()
    return nc


def _host_prep(x, W, bins, profile, goff, dt=ml_dtypes.bfloat16):
    """Build xt/wt for one core plus the (class -> (col, half)) map."""
    nbins = len(profile)
    prefn = np.concatenate([[0], np.cumsum(profile)]).astype(int)
    totn = int(prefn[-1])

    xt = np.zeros((128, nbins * KC * 128), dt)
    wt = np.zeros((128, KC * totn), dt)
    cols = np.empty(QCLS, np.int64)
    half = np.empty(QCLS, np.int64)

    xg_cache = {}

    def xgT(g):
        if g not in xg_cache:
            xg_cache[g] = np.ascontiguousarray(
                x[:, goff + g, :].astype(np.float32)
                .reshape(B, KC, 128).transpose(2, 1, 0)
            ).astype(dt)  # [128, KC, 64]
        return xg_cache[g]

    for j, bn in enumerate(bins):
        n = profile[j]
        coff = 0
        for hidx, (g, s, w) in enumerate(bn):
            xs = xgT(g)
            for k in range(KC):
                c0 = j * KC * 128 + k * 128 + hidx * 64
                xt[:, c0 : c0 + 64] = xs[:, k, :]
            wTk = W[s : s + w].astype(np.float32).reshape(w, KC, 128)
            for k in range(KC):
                wcol = KC * prefn[j] + k * n + coff
                wt[:, wcol : wcol + w] = wTk[:, k, :].T
            lc = np.arange(s, s + w) % QCLS
            cols[lc] = prefn[j] + coff + np.arange(w)
            half[lc] = hidx
            coff += w
    return {"xt": xt, "wt": wt}, cols, half


def kernel(x, co_W, cl_W, co_b, cl_b, co_group_of, cl_group_of, co_index,
           cl_index, group_len, _return_raw=False):
    x = np.asarray(x, np.float32)
    G = int(group_len)
    core_bins, profile, bounds = _plan(co_group_of, cl_group_of)

    key = ("v2", tuple(profile), tuple(bounds))
    if key not in _cache:
        _cache.clear()
        _cache[key] = _program(profile, bounds)
    nc = _cache[key]

    Ws = (np.asarray(co_W, np.float32)[0], np.asarray(cl_W, np.float32)[0])
    bs = (np.asarray(co_b, np.float32)[0], np.asarray(cl_b, np.float32)[0])
    in_maps, colmaps = [], []
    for k in range(8):
        bi, q = divmod(k, NQ)
        im, cols, half = _host_prep(x, Ws[bi], core_bins[k], profile, bi * G)
        in_maps.append(im)
        colmaps.append((cols, half))

    res = run_bass_kernel_spmd(nc, in_maps, list(range(8)))

    outs = []
    for bi, index in ((0, co_index), (1, cl_index)):
        full = np.empty((B, NC_CLS), np.float32)
        for q in range(NQ):
            core = bi * NQ + q
            cols, half = colmaps[core]
            oarr = np.asarray(res.results[core]["o"], np.float32)
            o3 = oarr.reshape(2, B, oarr.shape[1])
            vals = o3[half, :, cols]           # [QCLS, B]
            full[:, q * QCLS : (q + 1) * QCLS] = vals.T
        full += bs[bi][None, :]
        outs.append(full[:, np.asarray(index).astype(np.int64)])
    return outs[0], outs[1]
